# revision 10
# baseline (speedup 1.0000x reference)
"""Trainium2 Bass kernel for nn_Correlation: -mean(einsum('itj,itl->ijl', x, y)).

Math: mean over [B, C, C] of corr[b,j,l] = sum_t x[b,t,j] y[b,t,l] equals
  (1/(B*C^2)) * sum_{b,t} (sum_j x[b,t,j]) * (sum_l y[b,t,l])
so the kernel only needs per-row sums of x and y plus a dot product —
a pure memory-bound streaming reduction (no matmul).

Sharding: data-parallel over batch. 8 cores, 1 batch element each.
Each core streams its x[b], y[b] ([2048, 1024] f32, 8 MB each) through
SBUF in 1 MB chunks: x chunks load on the SP HWDGE ring and reduce on
the vector engine; y chunks load on the ACT ring and reduce on the
scalar engine (activation Copy with accum_out), so both reducers track
the DMA stream and the post-DMA tail is one small chunk's reduce. A
single elementwise multiply forms the [128, 16] per-row products, which
are stored (SWDGE) and summed/scaled on the host.

Constraints honored (this walrus build):
- HWDGE DIRECT2D DMAs: at most ONE sync wait -> every chunk gets a
  dedicated SBUF slot (no WAR/WAW waits); the store, which needs a DVE
  wait plus a completion-lane wait, goes via SWDGE (gpsimd).
- CTRL instructions: tail drain split into one drain per proc lane
  (_patch_tail_drain), since the stock single drain aggregates more
  waits than the ISA allows.
"""

import numpy as np

B, T, C = 8, 2048, 1024
P = 128           # SBUF partitions
RPP = T // P      # rows per partition overall (16)
AX = 2            # rows/partition per x-chunk (1 MB chunks)
AY = 2            # rows/partition per y-chunk
NCX = RPP // AX
NCY = RPP // AY
N_CORES = 8

_CACHE = {}


def _patch_tail_drain(tile):
    """Split TileContext's kernel-tail drain into one drain per proc lane.

    The stock tail emits a single SP Drain waiting on every outstanding
    sem (DVE + each DMA completion lane); this walrus build caps sync
    waits per CTRL instruction below that, so codegen fails with "Too
    many sync wait commands". Waiting on the sems one drain at a time is
    equivalent (SP program order) and keeps every instruction at 1 wait.
    """
    import re
    import bass_rust
    from concourse.vector_clock import ScopedClock

    if getattr(tile.TileContext, "_tail_drain_split", False):
        return

    def _drain_and_barrier(self, tick_clock, wait_clock):
        ticks = [int(s) for s in re.findall(r"-?\d+",
                                            repr(tick_clock.global_clock))]
        for i, t in enumerate(ticks):
            if t > 0:
                part = bass_rust.VectorClock(
                    [t if j == i else 0 for j in range(len(ticks))])
                d = self.nc.sync.drain()
                wait_clock.add_sem_waits(d.ins, ScopedClock({None: part}))
        self.nc.all_engine_barrier()
        assert self.sems is not None
        popped = self.nc._tile_sem_poison_stack.pop()
        assert popped is self._sem_poison
        self.nc.clear_and_free_semaphores(list(self.sems.allocated().values()))
        self.nc.all_engine_barrier()

    tile.TileContext._drain_and_barrier = _drain_and_barrier
    tile.TileContext._tail_drain_split = True


def _build_bass():
    import concourse.bass as bass
    import concourse.tile as tile
    from concourse import mybir

    _patch_tail_drain(tile)

    f32 = mybir.dt.float32
    nc = bass.Bass()
    x = nc.dram_tensor("x", [T, C], f32, kind="ExternalInput")
    y = nc.dram_tensor("y", [T, C], f32, kind="ExternalInput")
    outx = nc.dram_tensor("outx", [P, RPP], f32, kind="ExternalOutput")
    outy = nc.dram_tensor("outy", [P, RPP], f32, kind="ExternalOutput")

    xv = x[:].rearrange("(n p a) c -> n p a c", p=P, a=AX)
    yv = y[:].rearrange("(n p a) c -> n p a c", p=P, a=AY)

    with tile.TileContext(nc) as tc:
        with (
            # dedicated slot per chunk: load DMAs never carry WAR/WAW waits
            tc.tile_pool(name="iox", bufs=NCX) as iox,
            tc.tile_pool(name="ioy", bufs=NCY) as ioy,
            tc.tile_pool(name="acc", bufs=1) as acc,
        ):
            sxs = acc.tile([P, RPP], f32, tag="sxs")   # x row sums
            sys_ = acc.tile([P, RPP], f32, tag="sys")  # y row sums

            for ci in range(max(NCX, NCY)):
                if ci < NCX:
                    xt = iox.tile([P, AX, C], f32, tag="xt")
                    nc.sync.dma_start(out=xt[:], in_=xv[ci])
                    nc.vector.tensor_reduce(
                        out=sxs[:, ci * AX:(ci + 1) * AX], in_=xt[:],
                        axis=mybir.AxisListType.X, op=mybir.AluOpType.add,
                    )
                if ci < NCY:
                    yt = ioy.tile([P, AY, C], f32, tag="yt")
                    nc.scalar.dma_start(out=yt[:], in_=yv[ci])
                    for j in range(AY):
                        # in-place Copy: the main out is a dummy (only
                        # accum_out matters); writing back into the same
                        # slice avoids a scratch tile whose WAW reuse
                        # would add a second sync wait (ISA limit: 1).
                        nc.scalar.activation(
                            out=yt[:, j], in_=yt[:, j],
                            func=mybir.ActivationFunctionType.Copy,
                            accum_out=sys_[:, ci * AY + j:ci * AY + j + 1],
                        )

            # row-sum product happens on the host: an on-chip mul would
            # need waits on both DVE and ACT (> 1-wait ISA limit). SWDGE
            # stores tolerate multiple waits.
            nc.gpsimd.dma_start(out=outx[:], in_=sxs[:])
            nc.gpsimd.dma_start(out=outy[:], in_=sys_[:])
    return nc


def _run(x, y, trace=False):
    from concourse.bass_utils import run_bass_kernel_spmd

    if "nc" not in _CACHE:
        _CACHE["nc"] = _build_bass()
    nc = _CACHE["nc"]
    in_maps = [
        {"x": np.ascontiguousarray(x[i]), "y": np.ascontiguousarray(y[i])}
        for i in range(N_CORES)
    ]
    return run_bass_kernel_spmd(nc, in_maps, core_ids=list(range(N_CORES)),
                                trace=trace)


def kernel(**inputs) -> np.ndarray:
    x = np.asarray(inputs["x"], dtype=np.float32)
    y = np.asarray(inputs["y"], dtype=np.float32)
    res = _run(x, y, trace=False)
    s = 0.0
    for r in res.results:
        s += (r["outx"].astype(np.float64) * r["outy"].astype(np.float64)).sum()
    return np.array(-s / (B * C * C), dtype=np.float32)


# revision 13
# speedup vs baseline: 1.0489x; 1.0489x over previous
"""Trainium2 Bass kernel for nn_Correlation: -mean(einsum('itj,itl->ijl', x, y)).

Math: mean over [B, C, C] of corr[b,j,l] = sum_t x[b,t,j] y[b,t,l] equals
  (1/(B*C^2)) * sum_{b,t} (sum_j x[b,t,j]) * (sum_l y[b,t,l])
so the kernel only needs per-row sums of x and y plus a dot product —
a pure memory-bound streaming reduction (no matmul).

Sharding: data-parallel over batch. 8 cores, 1 batch element each.
Each core streams its x[b], y[b] ([2048, 1024] f32, 8 MB each) through
SBUF. All loads issue on the SP HWDGE ring (keeping the ACT sequencer
free) in descending chunk sizes — large chunks sustain HBM bandwidth,
small final chunks keep the post-stream reduce tail short. x chunks
reduce on the vector engine (free-dim tensor_reduce); y rows reduce on
the scalar engine (activation Copy with accum_out, written in place).
Row sums land in one [128, 2, 16] tile, stored via SWDGE; the host
multiplies x/y row sums, sums, and scales.

Constraints honored (this walrus build allows ONE sync wait per
instruction):
- every chunk gets a dedicated SBUF slot (no WAR/WAW waits on loads);
- activation writes in place (a scratch tile's WAW reuse would add a
  second wait);
- the final store needs DVE+ACT+lane waits, so it goes via SWDGE
  (gpsimd), whose lowering tolerates multiple waits;
- TileContext's tail drain is split into one drain per proc lane
  (_patch_tail_drain).
"""

import numpy as np

B, T, C = 8, 2048, 1024
P = 128             # SBUF partitions
RPP = T // P        # rows per partition (16)
CHUNKS = [6, 4, 3, 2, 1]   # rows/partition per chunk, sum = RPP
N_CORES = 8

_CACHE = {}


def _patch_tail_drain(tile):
    """Split TileContext's kernel-tail drain into one drain per proc lane.

    The stock tail emits a single SP Drain waiting on every outstanding
    sem (DVE + ACT + each DMA completion lane); this walrus build caps
    sync waits per instruction below that, so codegen fails with "Too
    many sync wait commands". Waiting on the sems one drain at a time is
    equivalent (SP program order) and keeps every instruction at 1 wait.
    """
    import re
    import bass_rust
    from concourse.vector_clock import ScopedClock

    if getattr(tile.TileContext, "_tail_drain_split", False):
        return

    def _drain_and_barrier(self, tick_clock, wait_clock):
        ticks = [int(s) for s in re.findall(r"-?\d+",
                                            repr(tick_clock.global_clock))]
        for i, t in enumerate(ticks):
            if t > 0:
                part = bass_rust.VectorClock(
                    [t if j == i else 0 for j in range(len(ticks))])
                d = self.nc.sync.drain()
                wait_clock.add_sem_waits(d.ins, ScopedClock({None: part}))
        self.nc.all_engine_barrier()
        assert self.sems is not None
        popped = self.nc._tile_sem_poison_stack.pop()
        assert popped is self._sem_poison
        self.nc.clear_and_free_semaphores(list(self.sems.allocated().values()))
        self.nc.all_engine_barrier()

    tile.TileContext._drain_and_barrier = _drain_and_barrier
    tile.TileContext._tail_drain_split = True


def _build_bass():
    import concourse.bass as bass
    import concourse.tile as tile
    from concourse import mybir

    _patch_tail_drain(tile)

    f32 = mybir.dt.float32
    nc = bass.Bass()
    x = nc.dram_tensor("x", [T, C], f32, kind="ExternalInput")
    y = nc.dram_tensor("y", [T, C], f32, kind="ExternalInput")
    out = nc.dram_tensor("out", [P, 2, RPP], f32, kind="ExternalOutput")

    with tile.TileContext(nc) as tc:
        with (
            # dedicated slot per chunk (unique tags, 1 buf each): load DMAs
            # never carry WAR/WAW waits
            tc.tile_pool(name="iox", bufs=1) as iox,
            tc.tile_pool(name="ioy", bufs=1) as ioy,
            tc.tile_pool(name="acc", bufs=1) as acc,
        ):
            sxy = acc.tile([P, 2, RPP], f32)  # [:,0,:] x sums, [:,1,:] y sums

            off = 0
            for a in CHUNKS:
                r0, r1 = off * P, (off + a) * P
                yt = ioy.tile([P, a, C], f32, tag=f"yt{off}")
                nc.sync.dma_start(
                    out=yt[:], in_=y[r0:r1, :].rearrange("(p a) c -> p a c", p=P))
                xt = iox.tile([P, a, C], f32, tag=f"xt{off}")
                nc.sync.dma_start(
                    out=xt[:], in_=x[r0:r1, :].rearrange("(p a) c -> p a c", p=P))

                nc.vector.tensor_reduce(
                    out=sxy[:, 0, off:off + a], in_=xt[:],
                    axis=mybir.AxisListType.X, op=mybir.AluOpType.add,
                )
                for j in range(a):
                    nc.scalar.activation(
                        out=yt[:, j], in_=yt[:, j],
                        func=mybir.ActivationFunctionType.Copy,
                        accum_out=sxy[:, 1, off + j:off + j + 1],
                    )
                off += a

            # two stores so each carries ONE wait (DVE half / ACT half)
            nc.gpsimd.dma_start(out=out[:, 0], in_=sxy[:, 0])
            nc.gpsimd.dma_start(out=out[:, 1], in_=sxy[:, 1])
    return nc


def _run(x, y, trace=False):
    from concourse.bass_utils import run_bass_kernel_spmd

    if "nc" not in _CACHE:
        _CACHE["nc"] = _build_bass()
    nc = _CACHE["nc"]
    in_maps = [
        {"x": np.ascontiguousarray(x[i]), "y": np.ascontiguousarray(y[i])}
        for i in range(N_CORES)
    ]
    return run_bass_kernel_spmd(nc, in_maps, core_ids=list(range(N_CORES)),
                                trace=trace)


def kernel(**inputs) -> np.ndarray:
    x = np.asarray(inputs["x"], dtype=np.float32)
    y = np.asarray(inputs["y"], dtype=np.float32)
    res = _run(x, y, trace=False)
    s = 0.0
    for r in res.results:
        o = r["out"].astype(np.float64)
        s += (o[:, 0, :] * o[:, 1, :]).sum()
    return np.array(-s / (B * C * C), dtype=np.float32)
